# revision 1
# baseline (speedup 1.0000x reference)
"""Trainium2 Bass kernel for a dense transformer block (nn_Block_78743930405073).

Block: x -> LN1 -> 16-head causal self-attention -> +x -> LN2 -> FFN(4096, ReLU) -> +.
Input x: [4, 2048, 1024] fp32.  8 NeuronCores, data-parallel over (batch, q-blocks).

Sharding: core c handles batch c//2.  The 16 query-blocks (128 rows each) of a
batch are split between the 2 cores of that batch in an interleaved pattern
(odd blocks / even blocks) so that both cores run the IDENTICAL program (SPMD)
with per-core data: slot j on every core processes one q-block over exactly
2j+2 key-blocks; causality differences between cores are handled by per-core
mask inputs applied to the last two key-blocks of each slot.

Precision: matmuls in bf16 (fp32 PSUM accumulation); LayerNorm statistics,
softmax normalization and the residual stream in fp32.
"""

import sys

if "/opt/trn_rl_repo" not in sys.path:
    sys.path.insert(0, "/opt/trn_rl_repo")

from contextlib import ExitStack

import ml_dtypes
import numpy as np

import concourse.bacc as bacc
import concourse.mybir as mybir
import concourse.tile as tile
from concourse import bass_utils

BF16 = mybir.dt.bfloat16
F32 = mybir.dt.float32
AF = mybir.ActivationFunctionType
AX = mybir.AxisListType

B, T, C = 4, 2048, 1024
NH, HD = 16, 64
FF = 4 * C
EPS = 1e-5
NB = T // 128          # 16 key/query blocks per batch
NSLOT = 8              # q-blocks per core
ROWS = NSLOT * 128     # 1024 own rows per core
NCORES = 8


def _own_blocks(half):
    # half 0 -> odd blocks {1,3,...,15}; half 1 -> even {0,2,...,14}.
    # slot j: trip count Tj = 2j+2 key-blocks on both cores.
    return [2 * j + 1 for j in range(NSLOT)] if half == 0 else [2 * j for j in range(NSLOT)]


def _trip(j):
    return 2 * j + 2


# ---------------------------------------------------------------- bass program


def _ln_tile(nc, pools, xa, h_out, use_act=False):
    """LayerNorm one [128, C] fp32 AP -> h_out [128, C] bf16 (pure normalize).

    The two big passes (square, normalize) run on ACT in exp-free regions
    (use_act=True) and on DVE where ACT is busy with softmax exp."""
    st = pools["stats"]
    ssum = st.tile([128, 1], F32, tag="ssum")
    ssq = st.tile([128, 1], F32, tag="ssq")
    sq = pools["sq"].tile([128, C], F32, tag="sq")
    nc.vector.reduce_sum(ssum[:], xa, axis=AX.X)
    nc.scalar.activation(sq[:], xa, AF.Square, accum_out=ssq[:])
    mu = st.tile([128, 1], F32, tag="mu")
    t0 = st.tile([128, 1], F32, tag="t0")
    var = st.tile([128, 1], F32, tag="var")
    std = st.tile([128, 1], F32, tag="std")
    rstd = st.tile([128, 1], F32, tag="rstd")
    nmr = st.tile([128, 1], F32, tag="nmr")
    nc.vector.tensor_scalar_mul(mu[:], ssum[:], 1.0 / C)
    nc.vector.tensor_scalar_mul(t0[:], ssq[:], 1.0 / C)
    nc.vector.tensor_mul(var[:], mu[:], mu[:])
    nc.vector.tensor_sub(var[:], t0[:], var[:])
    nc.vector.tensor_scalar_add(var[:], var[:], EPS)
    nc.scalar.activation(std[:], var[:], AF.Sqrt)
    nc.vector.reciprocal(rstd[:], std[:])
    nc.vector.tensor_mul(nmr[:], mu[:], rstd[:])
    nc.vector.tensor_scalar_mul(nmr[:], nmr[:], -1.0)
    nc.scalar.activation(h_out, xa, AF.Identity, bias=nmr[:], scale=rstd[:])


def _pe_transpose(nc, trp, ident, dst3, src, tslice, engine):
    """Transpose src [128, C] bf16 into dst3[:, cc, tslice] via PE.

    4 blocks transpose into one half-bank [128,512] bf16 PSUM tile, then one
    wide strided copy evacuates them."""
    for g in range(2):
        tr = trp.tile([128, 512], BF16, tag="tr")
        tr3 = tr[:].rearrange("p (a t) -> p a t", a=4)
        for cc in range(4):
            nc.tensor.transpose(tr3[:, cc, :], src[:, (4 * g + cc) * 128:(4 * g + cc + 1) * 128],
                                ident[:])
        if engine == "act":
            nc.scalar.activation(dst3[:, 4 * g:4 * g + 4, tslice], tr3, AF.Copy)
        else:
            nc.vector.tensor_copy(dst3[:, 4 * g:4 * g + 4, tslice], tr3)


def build_program():
    nc = bacc.Bacc("TRN2", target_bir_lowering=False, debug=False)

    d = {}
    d["x_full"] = nc.dram_tensor("x_full", [T, C], F32, kind="ExternalInput")
    d["wq"] = nc.dram_tensor("wq", [C, C], BF16, kind="ExternalInput")
    d["wk"] = nc.dram_tensor("wk", [C, C], BF16, kind="ExternalInput")
    d["wv"] = nc.dram_tensor("wv", [C, C], BF16, kind="ExternalInput")
    d["wo"] = nc.dram_tensor("wo", [C + 128, C], BF16, kind="ExternalInput")
    d["w1"] = nc.dram_tensor("w1", [C, FF], BF16, kind="ExternalInput")
    d["w2"] = nc.dram_tensor("w2", [FF + 128, C], BF16, kind="ExternalInput")
    d["bq"] = nc.dram_tensor("bq", [C], F32, kind="ExternalInput")
    d["bk"] = nc.dram_tensor("bk", [C], F32, kind="ExternalInput")
    d["b1"] = nc.dram_tensor("b1", [FF], F32, kind="ExternalInput")
    d["masks"] = nc.dram_tensor("masks", [128, NSLOT * 2 * 128], BF16, kind="ExternalInput")
    d["out_own"] = nc.dram_tensor("out_own", [ROWS, C], F32, kind="ExternalOutput")

    with tile.TileContext(nc) as tc:
        _emit(nc, tc, d)
    nc.compile()
    return nc


def _attn_slot(nc, j, kts3, qts3, vps3, msk3, pools):
    """Attention for slot j (all 16 heads) -> y_sb row-major [128q, C] bf16."""
    tj = _trip(j)
    c = j // 2
    spool, ypsum, apool, ypool, rpool = (pools["spool"], pools["ypsum"],
                                         pools["apool"], pools["ypool"], pools["rpool"])
    y_sb = ypool.tile([128, C], BF16, tag="y")
    for h in range(NH):
        pb, hb = 64 * (h % 2), h // 2
        qth = qts3[c][pb:pb + 64, hb, (j % 2) * 128:(j % 2) * 128 + 128]
        py = ypsum.tile([128, 65], F32, tag="py")
        ngrp = (tj + 3) // 4
        for g in range(ngrp):
            w = min(4, tj - g * 4)
            ps = spool.tile([128, 512], F32, tag="ss")
            for kk in range(w):
                kb = g * 4 + kk
                nc.tensor.matmul(ps[:, kk * 128:(kk + 1) * 128],
                                 kts3[kb // 4][pb:pb + 64, hb, (kb % 4) * 128:(kb % 4) * 128 + 128],
                                 qth, start=True, stop=True)
            ag = apool.tile([128, 512], BF16, tag="ag")
            nc.scalar.activation(ag[:, 0:w * 128], ps[:, 0:w * 128], AF.Exp, scale=0.125)
            for kk in range(w):
                kb = g * 4 + kk
                if kb >= tj - 2:
                    m = kb - (tj - 2)
                    nc.vector.tensor_mul(ag[:, kk * 128:(kk + 1) * 128],
                                         ag[:, kk * 128:(kk + 1) * 128],
                                         msk3[:, 2 * j + m, :])
                nc.tensor.matmul(py[:], ag[:, kk * 128:(kk + 1) * 128],
                                 vps3[kb][:, h, :], start=(kb == 0), stop=(kb == tj - 1))
        rinv = rpool.tile([128, 1], F32, tag="r")
        nc.vector.reciprocal(rinv[:], py[:, 64:65])
        nc.vector.tensor_scalar_mul(y_sb[:, h * 64:(h + 1) * 64], py[:, 0:64], rinv[:])
    return y_sb


def _emit(nc, tc, d):
    with ExitStack() as outer:
        stat = outer.enter_context(tc.tile_pool(name="static", bufs=1))
        ones = stat.tile([128, 128], BF16, tag="ones")        # row 0 = 1.0
        bqt = stat.tile([128, 8], F32, tag="bqt")
        bkt = stat.tile([128, 8], F32, tag="bkt")
        b1t = stat.tile([128, 32], F32, tag="b1t")
        nc.gpsimd.memset(ones[:], 0.0)
        nc.gpsimd.memset(ones[0:1, :], 1.0)
        nc.scalar.dma_start(bqt[:], d["bq"].ap().rearrange("(a p) -> p a", p=128))
        nc.scalar.dma_start(bkt[:], d["bk"].ap().rearrange("(a p) -> p a", p=128))
        nc.scalar.dma_start(b1t[:], d["b1"].ap().rearrange("(a p) -> p a", p=128))

        pools = {}
        pools["stats"] = outer.enter_context(tc.tile_pool(name="stats", bufs=4))
        pools["sq"] = outer.enter_context(tc.tile_pool(name="sq", bufs=2))

        # yt slot tiles live B -> C: right side heap
        ytp = outer.enter_context(tc.tile_pool(name="ytp", bufs=1))
        yts = []
        for j in range(NSLOT):
            yt_j = ytp.tile([128, 8 * 128], BF16, tag=f"yt{j}", name=f"yt{j}")
            yts.append(yt_j[:].rearrange("p (a t) -> p a t", a=8))

        # ============ Phases A+B interleaved, per 512-token chunk ============
        with ExitStack() as phab:
            abp = phab.enter_context(tc.tile_pool(name="ab", bufs=1))
            msk = abp.tile([128, NSLOT * 2 * 128], BF16, tag="msk")
            msk3 = msk[:].rearrange("p (s q) -> p s q", s=NSLOT * 2)
            nc.scalar.dma_start(msk[:], d["masks"].ap())
            kts3, qts3, vps3 = [], [], []
            for c in range(4):
                ktc = abp.tile([128, 8 * 512], BF16, tag=f"kt{c}", name=f"kt{c}")
                kts3.append(ktc[:].rearrange("p (a t) -> p a t", a=8))
                qtc = abp.tile([128, 8 * 256], BF16, tag=f"qt{c}", name=f"qt{c}")
                qts3.append(qtc[:].rearrange("p (a t) -> p a t", a=8))
            for gt in range(NB):
                vpt = abp.tile([128, NH * 65], BF16, tag=f"vp{gt}", name=f"vp{gt}")
                v3 = vpt[:].rearrange("p (h e) -> p h e", h=NH)
                nc.gpsimd.memset(v3[:, :, 64:65], 1.0)
                vps3.append(v3)

            wpool = phab.enter_context(tc.tile_pool(name="wqkv", bufs=1))
            wq_sb = wpool.tile([128, 8 * C], BF16, tag="wq")
            wk_sb = wpool.tile([128, 8 * C], BF16, tag="wk")
            wv_sb = wpool.tile([128, 8 * C], BF16, tag="wv")
            wq3 = wq_sb[:].rearrange("p (a c) -> p a c", a=8)
            wk3 = wk_sb[:].rearrange("p (a c) -> p a c", a=8)
            wv3 = wv_sb[:].rearrange("p (a c) -> p a c", a=8)
            nc.scalar.dma_start(wk3, d["wk"].ap().rearrange("(a p) c -> p a c", p=128))
            nc.scalar.dma_start(wv3, d["wv"].ap().rearrange("(a p) c -> p a c", p=128))
            nc.scalar.dma_start(wq3, d["wq"].ap().rearrange("(a p) c -> p a c", p=128))

            htp = phab.enter_context(tc.tile_pool(name="ht", bufs=2))
            xpool = phab.enter_context(tc.tile_pool(name="xa", bufs=3))
            hpool = phab.enter_context(tc.tile_pool(name="hstage", bufs=3))
            pps = phab.enter_context(tc.tile_pool(name="ppsum", bufs=2, space="PSUM"))
            pools["spool"] = phab.enter_context(tc.tile_pool(name="spsum", bufs=3, space="PSUM"))
            pools["ypsum"] = phab.enter_context(tc.tile_pool(name="ypsum", bufs=2, space="PSUM"))
            pools["apool"] = phab.enter_context(tc.tile_pool(name="atile", bufs=3))
            pools["ypool"] = phab.enter_context(tc.tile_pool(name="ysb", bufs=2))
            pools["rpool"] = phab.enter_context(tc.tile_pool(name="rinv", bufs=4))

            for c in range(4):
                ht = htp.tile([128, 8 * 512], BF16, tag="ht")
                ht3 = ht[:].rearrange("p (a t) -> p a t", a=8)
                for tt in range(4):
                    xa = xpool.tile([128, C], F32, tag="xa")
                    nc.sync.dma_start(
                        xa[:], d["x_full"].ap()[(c * 4 + tt) * 128:(c * 4 + tt + 1) * 128, :])
                    hst = hpool.tile([128, C], BF16, tag="h")
                    _ln_tile(nc, pools, xa[:], hst[:], use_act=(c < 2))
                    for cc in range(8):
                        nc.sync.dma_start_transpose(
                            ht3[:, cc, tt * 128:(tt + 1) * 128],
                            hst[:, cc * 128:(cc + 1) * 128])
                # K projection
                for kc in range(8):
                    ps = pps.tile([128, 512], F32, tag="pp")
                    for cin in range(8):
                        nc.tensor.matmul(ps[:], wk3[:, cin, kc * 128:(kc + 1) * 128],
                                         ht3[:, cin, :], start=(cin == 0), stop=(cin == 7))
                    nc.vector.tensor_scalar_add(kts3[c][:, kc, :], ps[:],
                                                bkt[:, kc:kc + 1])
                # V projection (row-major)
                for tt in range(4):
                    gt = c * 4 + tt
                    for hh in range(2):
                        ps = pps.tile([128, 512], F32, tag="pp")
                        for cin in range(8):
                            nc.tensor.matmul(ps[:], ht3[:, cin, tt * 128:(tt + 1) * 128],
                                             wv3[:, cin, hh * 512:(hh + 1) * 512],
                                             start=(cin == 0), stop=(cin == 7))
                        nc.vector.tensor_copy(vps3[gt][:, hh * 8:(hh + 1) * 8, 0:64],
                                              ps[:].rearrange("p (h e) -> p h e", h=8))
                # Q projection: own blocks at even in-chunk positions {0, 2}
                for qc in range(8):
                    ps = pps.tile([128, 512], F32, tag="pp")
                    for cin in range(8):
                        rr = ht3[:, cin, :].rearrange("p (s e t) -> p s e t", s=2, e=2)[:, :, 0, :]
                        nc.tensor.matmul(ps[:, 0:256], wq3[:, cin, qc * 128:(qc + 1) * 128],
                                         rr, start=(cin == 0), stop=(cin == 7))
                    nc.vector.tensor_scalar_add(qts3[c][:, qc, :], ps[:, 0:256],
                                                bqt[:, qc:qc + 1])
                # attention for the two slots whose keys are now complete
                for j in (2 * c, 2 * c + 1):
                    y_sb = _attn_slot(nc, j, kts3, qts3, vps3, msk3, pools)
                    for cc in range(8):
                        nc.sync.dma_start_transpose(
                            yts[j][:, cc, :], y_sb[:, cc * 128:(cc + 1) * 128])

        # =============== Phase C: out-proj + residual, LN2 ===============
        with ExitStack() as pcd:  # x2/h2t live C -> D
            midp = pcd.enter_context(tc.tile_pool(name="mid", bufs=1))
            x2 = midp.tile([128, 8 * C], F32, tag="x2")
            x23 = x2[:].rearrange("p (a c) -> p a c", a=8)
            h2t = midp.tile([128, 8 * ROWS], BF16, tag="h2t")
            h2t3 = h2t[:].rearrange("p (a t) -> p a t", a=8)

            with ExitStack() as phc:
                wpool = phc.enter_context(tc.tile_pool(name="wo", bufs=1))
                wo_sb = wpool.tile([128, 9 * C], BF16, tag="wo")
                wo3 = wo_sb[:].rearrange("p (a c) -> p a c", a=9)
                nc.scalar.dma_start(wo3, d["wo"].ap().rearrange("(a p) c -> p a c", p=128))
                xrp = phc.enter_context(tc.tile_pool(name="xres", bufs=3))
                pps = phc.enter_context(tc.tile_pool(name="opsum", bufs=4, space="PSUM"))
                hpool = phc.enter_context(tc.tile_pool(name="h2stage", bufs=2))

                for ts in range(8):
                    for cc in range(2):
                        ps = pps.tile([128, 512], F32, tag="op")
                        for yc in range(8):
                            nc.tensor.matmul(ps[:], yts[ts][:, yc, :],
                                             wo3[:, yc, cc * 512:(cc + 1) * 512],
                                             start=(yc == 0), stop=False)
                        nc.tensor.matmul(ps[:], ones[:],
                                         wo3[:, 8, cc * 512:(cc + 1) * 512],
                                         start=False, stop=True)
                        # residual: own rows are x_full's even permuted blocks
                        xr = xrp.tile([128, 512], F32, tag="xr")
                        nc.sync.dma_start(
                            xr[:], d["x_full"].ap()[2 * ts * 128:(2 * ts + 1) * 128,
                                                    cc * 512:(cc + 1) * 512])
                        nc.vector.tensor_add(x23[:, ts, cc * 512:(cc + 1) * 512], ps[:], xr[:])
                    hst = hpool.tile([128, C], BF16, tag="h2")
                    _ln_tile(nc, pools, x23[:, ts, :], hst[:], use_act=False)
                    for cc in range(8):
                        nc.sync.dma_start_transpose(
                            h2t3[:, cc, ts * 128:(ts + 1) * 128],
                            hst[:, cc * 128:(cc + 1) * 128])

            # =============== Phase D: FFN ===============
            with ExitStack() as phd:
                atp = phd.enter_context(tc.tile_pool(name="at", bufs=1))
                ats = []
                for f in range(33):
                    at_f = atp.tile([128, ROWS], BF16, tag=f"at{f}", name=f"at{f}")
                    ats.append(at_f)
                nc.gpsimd.memset(ats[32][:], 0.0)
                nc.gpsimd.memset(ats[32][0:1, :], 1.0)

                w1p = phd.enter_context(tc.tile_pool(name="w1s", bufs=4))
                pps = phd.enter_context(tc.tile_pool(name="fpsum", bufs=3, space="PSUM"))
                d_w1r = d["w1"].ap().rearrange("(a p) f -> p a f", p=128)
                for f in range(32):
                    w1t = w1p.tile([128, 8 * 128], BF16, tag="w1t")
                    w1t3 = w1t[:].rearrange("p (a t) -> p a t", a=8)
                    nc.sync.dma_start(w1t3, d_w1r[:, :, f * 128:(f + 1) * 128])
                    for chunk in range(2):
                        ps = pps.tile([128, 512], F32, tag="fp")
                        for cin in range(8):
                            nc.tensor.matmul(ps[:], w1t3[:, cin, :],
                                             h2t3[:, cin, chunk * 512:(chunk + 1) * 512],
                                             start=(cin == 0), stop=(cin == 7))
                        nc.scalar.activation(ats[f][:, chunk * 512:(chunk + 1) * 512],
                                             ps[:], AF.Relu, bias=b1t[:, f:f + 1])

                w2p = phd.enter_context(tc.tile_pool(name="w2s", bufs=36))
                outp = phd.enter_context(tc.tile_pool(name="outs", bufs=3))
                d_w2r = d["w2"].ap().rearrange("(a p) c -> p a c", p=128)
                for cc in range(2):
                    w2ts = []
                    for f in range(33):
                        w2t = w2p.tile([128, 512], BF16, tag="w2t")
                        nc.sync.dma_start(w2t[:], d_w2r[:, f, cc * 512:(cc + 1) * 512])
                        w2ts.append(w2t)
                    for ts in range(8):
                        ps = pps.tile([128, 512], F32, tag="fp2")
                        for f in range(33):
                            nc.tensor.matmul(ps[:], ats[f][:, ts * 128:(ts + 1) * 128],
                                             w2ts[f][:], start=(f == 0), stop=(f == 32))
                        ot = outp.tile([128, 512], F32, tag="ot")
                        nc.vector.tensor_add(ot[:], ps[:],
                                             x23[:, ts, cc * 512:(cc + 1) * 512])
                        nc.sync.dma_start(
                            d["out_own"].ap()[ts * 128:(ts + 1) * 128,
                                              cc * 512:(cc + 1) * 512], ot[:])


# ---------------------------------------------------------------- host side

_NC_CACHE = None


def _get_nc():
    global _NC_CACHE
    if _NC_CACHE is None:
        _NC_CACHE = build_program()
    return _NC_CACHE


def _bf16(a):
    return np.asarray(a, dtype=np.float32).astype(ml_dtypes.bfloat16)


def make_in_maps(x, Wq, Wk, Wv, Wo, bo, W1, b1, W2, b2, g1, be1, g2, be2):
    x = np.asarray(x, dtype=np.float32)
    g1 = np.asarray(g1, np.float32); be1 = np.asarray(be1, np.float32)
    g2 = np.asarray(g2, np.float32); be2 = np.asarray(be2, np.float32)
    Wq = np.asarray(Wq, np.float32); Wk = np.asarray(Wk, np.float32)
    Wv = np.asarray(Wv, np.float32); Wo = np.asarray(Wo, np.float32)
    W1 = np.asarray(W1, np.float32); W2 = np.asarray(W2, np.float32)
    bo = np.asarray(bo, np.float32); b1 = np.asarray(b1, np.float32)
    b2 = np.asarray(b2, np.float32)

    wq_e = _bf16(g1[:, None] * Wq)
    wk_e = _bf16(g1[:, None] * Wk)
    wv_e = _bf16(g1[:, None] * Wv)
    bq = (be1 @ Wq).astype(np.float32)
    bk = (be1 @ Wk).astype(np.float32)
    bv = (be1 @ Wv).astype(np.float32)
    # softmax rows sum to 1 => y_h = (sm @ V_h) + bv_h; fold bv@Wo into bo.
    bo_eff = (bo + bv @ Wo).astype(np.float32)
    wo_pad = np.zeros((C + 128, C), np.float32)
    wo_pad[:C] = Wo
    wo_pad[C] = bo_eff
    wo_pad = _bf16(wo_pad)
    w1_e = _bf16(g2[:, None] * W1)
    b1v = (be2 @ W1 + b1).astype(np.float32)
    w2_pad = np.zeros((FF + 128, C), np.float32)
    w2_pad[:FF] = W2
    w2_pad[FF] = b2
    w2_pad = _bf16(w2_pad)

    tri = np.triu(np.ones((128, 128), np.float32))  # [k, q]: keep k <= q
    in_maps = []
    for core in range(NCORES):
        b, half = core // 2, core % 2
        own = _own_blocks(half)
        other = _own_blocks(1 - half)
        # permuted block order: own blocks at even positions
        perm = []
        for j in range(NSLOT):
            perm.append(own[j])
            perm.append(other[j])
        # perm[p] = original block at permuted position p
        x_perm = np.concatenate([x[b, g * 128:(g + 1) * 128, :] for g in perm], axis=0)
        # masks: slot j (own block g=own[j], orig row range [128g, 128g+128))
        # attends permuted key blocks 0..Tj-1; mask on the last two.
        masks = np.zeros((NSLOT, 2, 128, 128), np.float32)
        for j in range(NSLOT):
            tj = _trip(j)
            g = own[j]
            q_orig = g * 128 + np.arange(128)          # original query rows
            for m in range(2):
                kb = tj - 2 + m                        # permuted key block idx
                k_orig = perm[kb] * 128 + np.arange(128)
                masks[j, m] = (k_orig[:, None] <= q_orig[None, :]).astype(np.float32)
        masks_t = _bf16(np.transpose(masks, (2, 0, 1, 3)).reshape(128, NSLOT * 2 * 128))
        in_maps.append({
            "x_full": np.ascontiguousarray(x_perm),
            "wq": wq_e, "wk": wk_e, "wv": wv_e, "wo": wo_pad,
            "w1": w1_e, "w2": w2_pad,
            "bq": bq, "bk": bk, "b1": b1v,
            "masks": masks_t,
        })
    return in_maps


def scatter_out(results):
    out = np.empty((B, T, C), np.float32)
    for core in range(NCORES):
        b, half = core // 2, core % 2
        own = _own_blocks(half)
        oo = results[core]["out_own"]
        for j, g in enumerate(own):
            out[b, g * 128:(g + 1) * 128, :] = oo[j * 128:(j + 1) * 128, :]
    return out


def kernel(**inputs):
    nc = _get_nc()
    in_maps = make_in_maps(**inputs)
    res = bass_utils.run_bass_kernel_spmd(nc, in_maps, core_ids=list(range(NCORES)))
    return scatter_out(res.results)

